# revision 53
# baseline (speedup 1.0000x reference)
"""Trainium2 Bass kernel for sliding-window causal self-attention (GQA + RoPE +
RMS-norm QK + value-embedding gating).

Sharding: 8 cores = 2 (batch) x 4 (KV groups).  Each core handles one batch
element and one KV head (= 4 query heads), computes a partial output through
the row-slice of Wproj for its heads; the host sums the 4 partials per batch.

v3: DMA front-loading + PE warmup (HAM un-throttle), K pre-normalized in
phase 1 so the attention exp has uniform scale and merges into 2 wide ACT
instructions per q-tile reading a single 5-bank PSUM score arena, lagged
attention pipeline (scores/exp one q-tile ahead of PV; proj two behind)
keeping the PE stream dense, mask/cast work spread to the GpSimd engine.
"""

import sys
import os

for _p in ("/root/.axon_site", "/root/.axon_site/_ro/trn_rl_repo",
           "/root/.axon_site/_ro/pypackages", "/opt/trn_rl_repo"):
    if os.path.isdir(_p) and _p not in sys.path:
        sys.path.append(_p)

import numpy as np
import ml_dtypes
from contextlib import ExitStack

import concourse.bass as bass
import concourse.tile as tile
from concourse import bacc, mybir
from concourse.bass_utils import run_bass_kernel_spmd

BF16 = ml_dtypes.bfloat16
N_HEAD, N_KV, HEAD_DIM, WINDOW, N_EMBD = 16, 4, 64, 512, 1024
B, T = 2, 2048
NCORES = 8
TCH = 512               # token chunk for the projection phase
NCH = T // TCH          # 4
NTT = T // 128          # 16 t-tiles

F32 = mybir.dt.float32
BF = mybir.dt.bfloat16
AF = mybir.ActivationFunctionType
OP = mybir.AluOpType

TWO_PSUM = False        # DVE tensor ops reading two PSUM operands
WIDE_EXP = False        # ACT exp reads spanning multiple PSUM banks

_cache = {}


def _build():
    nc = bacc.Bacc("TRN2", target_bir_lowering=False, debug=False,
                   num_devices=NCORES)

    xt_d = nc.dram_tensor("xt", [8, 128, T], BF, kind="ExternalInput")
    wq_d = nc.dram_tensor("wq", [128, 8 * 256], BF, kind="ExternalInput")
    wkv_d = nc.dram_tensor("wkv", [128, 8 * 128], BF, kind="ExternalInput")
    wg_d = nc.dram_tensor("wg", [32, 1], BF, kind="ExternalInput")
    wp_d = nc.dram_tensor("wp", [128, 2 * 1024], BF, kind="ExternalInput")
    cs1_d = nc.dram_tensor("cs1", [128, T], BF, kind="ExternalInput")
    cs2_d = nc.dram_tensor("cs2", [128, T], BF, kind="ExternalInput")
    ve_d = nc.dram_tensor("ve2", [128, 16 * 64], BF, kind="ExternalInput")
    msk_d = nc.dram_tensor("masks", [128, 1024], BF, kind="ExternalInput")
    id_d = nc.dram_tensor("ident", [64, 64], BF, kind="ExternalInput")
    selq_d = nc.dram_tensor("selq2", [128, 2], BF, kind="ExternalInput")
    selw_d = nc.dram_tensor("selw", [33, 128], BF, kind="ExternalInput")
    on64_d = nc.dram_tensor("ones64", [64, 1], BF, kind="ExternalInput")
    selb_d = nc.dram_tensor("selb", [128, 33], BF, kind="ExternalInput")
    onw_d = nc.dram_tensor("onw", [65, 64], BF, kind="ExternalInput")
    on1x_d = nc.dram_tensor("ones1x64", [1, 64], BF, kind="ExternalInput")
    mab_d = nc.dram_tensor("mab", [128, 256], BF, kind="ExternalInput")
    out_d = nc.dram_tensor("out", [T, N_EMBD], BF, kind="ExternalOutput")

    with tile.TileContext(nc) as tc, ExitStack() as ctx:
        pers = ctx.enter_context(tc.tile_pool(name="pers", bufs=1))
        wk = ctx.enter_context(tc.tile_pool(name="wk", bufs=8))
        wfp = ctx.enter_context(tc.tile_pool(name="wfp", bufs=4))
        sm = ctx.enter_context(tc.tile_pool(name="sm", bufs=4))
        ow = ctx.enter_context(tc.tile_pool(name="ow", bufs=4))
        evp = ctx.enter_context(tc.tile_pool(name="evp", bufs=20))
        sqp = ctx.enter_context(tc.tile_pool(name="sqp", bufs=4))
        ptp = ctx.enter_context(tc.tile_pool(name="ptp", bufs=2))
        # PSUM: score tiles (5 banks) + yx (1 bank) + pj pool (2 banks)
        pST = ctx.enter_context(tc.tile_pool(name="pST", bufs=2, space="PSUM"))
        pSS = ctx.enter_context(tc.tile_pool(name="pSS", bufs=1, space="PSUM"))
        pYX = ctx.enter_context(tc.tile_pool(name="pYX", bufs=1, space="PSUM"))
        pPJ = ctx.enter_context(tc.tile_pool(name="pPJ", bufs=2, space="PSUM"))

        # ---- DMA front-loading: ident first (PE warmup dep), then xt/w
        # in first-use order; small aux tensors go on the vector queue ----
        id_sb = pers.tile([64, 64], BF, tag="ident")
        nc.sync.dma_start(id_sb[:], id_d[:])
        xt_sb = [None] * 8
        for k, eng in ((0, nc.sync), (1, nc.gpsimd), (2, nc.scalar)):
            xt_sb[k] = pers.tile([128, T], BF, tag=f"xt{k}", name=f"xt{k}")
        wq_sb = pers.tile([128, 8 * 256], BF, tag="wq")
        nc.scalar.dma_start(wq_sb[:], wq_d[:])
        cs1_sb = pers.tile([128, T], BF, tag="cs1")
        nc.gpsimd.dma_start(cs1_sb[:], cs1_d[:])
        nc.sync.dma_start(xt_sb[0][:], xt_d[0])
        nc.gpsimd.dma_start(xt_sb[1][:], xt_d[1])
        nc.scalar.dma_start(xt_sb[2][:], xt_d[2])
        for k, eng in ((3, nc.sync), (4, nc.gpsimd), (5, nc.scalar),
                       (6, nc.sync), (7, nc.gpsimd)):
            t_ = pers.tile([128, T], BF, tag=f"xt{k}", name=f"xt{k}")
            eng.dma_start(t_[:], xt_d[k])
            xt_sb[k] = t_
        wkv_sb = pers.tile([128, 8 * 128], BF, tag="wkv")
        nc.scalar.dma_start(wkv_sb[:], wkv_d[:])
        selq_sb = pers.tile([128, 2], BF, tag="selq2")
        nc.sync.dma_start(selq_sb[:], selq_d[:])
        selw_sb = pers.tile([33, 128], BF, tag="selw")
        nc.sync.dma_start(selw_sb[:], selw_d[:])
        on64_sb = pers.tile([64, 1], BF, tag="on64")
        nc.sync.dma_start(on64_sb[:], on64_d[:])
        selb_sb = pers.tile([128, 33], BF, tag="selb")
        nc.sync.dma_start(selb_sb[:], selb_d[:])
        onw_sb = pers.tile([65, 64], BF, tag="onw")
        nc.sync.dma_start(onw_sb[:], onw_d[:])
        on1x_sb = pers.tile([1, 64], BF, tag="on1x")
        nc.sync.dma_start(on1x_sb[:], on1x_d[:])
        wg_sb = pers.tile([32, 1], BF, tag="wg")
        nc.sync.dma_start(wg_sb[:], wg_d[:])
        mab_sb_ = pers.tile([128, 256], BF, tag="mab")
        nc.sync.dma_start(mab_sb_[:], mab_d[:])
        ma_sb = mab_sb_[:, 0:128]
        mb_sb = mab_sb_[:, 128:256]
        cs2_sb = pers.tile([128, T], BF, tag="cs2")
        nc.gpsimd.dma_start(cs2_sb[:], cs2_d[:])
        ve_sb = pers.tile([128, 16 * 64], BF, tag="ve")
        nc.sync.dma_start(ve_sb[:], ve_d[:])
        mskc_sb = pers.tile([128, 512], BF, tag="mskc")
        nc.sync.dma_start(mskc_sb[:], msk_d[:, 0:512])
        mskw_sb = pers.tile([128, 512], BF, tag="mskw")
        nc.sync.dma_start(mskw_sb[:], msk_d[:, 512:1024])
        wp_sb = pers.tile([128, 2 * 1024], BF, tag="wp")
        nc.scalar.dma_start(wp_sb[:], wp_d[:])

        # ---- persistent intermediates ----
        # Q^T, 4 heads side-by-side per q-tile: [64, qt(16) x h(4) x 128]
        q4t = pers.tile([64, NTT * 512], BF, tag="q4t")
        kt_sb = pers.tile([64, T], BF, tag="kt")      # K^T (pre-normalized)
        vn_sb = pers.tile([128, NTT * 65], BF, tag="vn")  # V natural + ones col
        # y^T: [128 (2 heads stacked), p(2) x T]
        yt_sb = pers.tile([128, 2 * T], BF, tag="yt")
        g_sb = pers.tile([128, NTT], F32, tag="g")    # sigmoid gates, natural

        nc.vector.memset(vn_sb[:], 1.0)      # ones columns (col 64 of each group)
        biasq_sb = pers.tile([33, 1], F32, tag="biasq")
        nc.vector.memset(biasq_sb[:], 64e-6)
        biask_sb = pers.tile([1, 1], F32, tag="biask")
        nc.vector.memset(biask_sb[:], 1e-6)

        # ---- PE warmup during initial DMA (HAM un-throttle); depends only
        # on the tiny ident tensor which is the first DMA issued ----
        warm = pPJ.tile([128, TCH], F32, tag="pj", name="warm")
        for _ in range(40):
            nc.tensor.matmul(warm[0:64, 0:64], id_sb[:], id_sb[:],
                             start=True, stop=True)

        def emit_gates():
            # gates for all t-tiles (sigmoid via exp + reciprocal)
            gps = pPJ.tile([128, NTT], F32, tag="pj", name="gps")
            for tt in range(NTT):
                nc.tensor.matmul(gps[:, tt:tt + 1],
                                 xt_sb[0][0:32, tt * 128:(tt + 1) * 128],
                                 wg_sb[:], start=True, stop=True)
            eg = sm.tile([128, NTT], F32, tag="u", name="eg")
            nc.scalar.activation(eg[:], gps[:], AF.Exp, scale=-1.0)
            eg1 = sm.tile([128, NTT], F32, tag="u", name="eg1")
            nc.vector.tensor_scalar_add(eg1[:], eg[:], 1.0)
            nc.vector.reciprocal(g_sb[:], eg1[:])

        def qkv_matmul(psum, w_sb, col0, ncol, c0):
            for k in range(8):
                nc.tensor.matmul(
                    psum, w_sb[:, k * ncol + col0: k * ncol + col0 + 128],
                    xt_sb[k][:, c0:c0 + TCH],
                    start=(k == 0), stop=(k == 7))

        def chunk_mms(ch):
            """QKV matmuls (dense PE block) + immediate PSUM evacuation."""
            c0 = ch * TCH
            psqw = pST.tile([128, 2 * TCH], F32, tag="st", name="psqw")
            psq0 = psqw[:, 0:512]
            qkv_matmul(psq0, wq_sb, 0, 256, c0)
            psq1 = psqw[:, 512:1024]
            qkv_matmul(psq1, wq_sb, 128, 256, c0)
            pskv = pSS.tile([128, TCH], F32, tag="ss", name="pskv")
            qkv_matmul(pskv[:], wkv_sb, 0, 128, c0)
            tl = {}
            sq = sqp.tile([128, 1024], BF, tag="sq", name="sq")
            nc.scalar.square(sq[:, 0:512], psq0)
            nc.scalar.square(sq[:, 512:1024], psq1)
            pb0 = evp.tile([128, TCH], BF, tag="e", name="pb0")
            nc.scalar.copy(pb0[:], psq0)
            pb1 = evp.tile([128, TCH], BF, tag="e", name="pb1")
            nc.vector.tensor_copy(pb1[:], psq1)
            sqk = evp.tile([64, TCH], BF, tag="e", name="sqk")
            nc.scalar.square(sqk[:], pskv[0:64])
            kb = evp.tile([64, TCH], BF, tag="e", name="kb")
            nc.vector.tensor_copy(kb[:], pskv[0:64])
            vt = evp.tile([64, TCH], BF, tag="e", name="vt")
            nc.vector.tensor_copy(vt[:], pskv[64:128])
            tl.update(sq=sq, pb0=pb0, pb1=pb1, sqk=sqk, kb=kb, vt=vt)
            return tl

        def epilogue_stages(ch, tl):
            """Four emit-stages of the per-chunk rms/rope/V epilogue; the
            cross-engine chain between consecutive stages resolves while a
            full attention block sits between them in the engine queues."""
            c0 = ch * TCH
            csl = slice(c0, c0 + TCH)
            q4v = q4t[:, ch * 2048:(ch + 1) * 2048].rearrange(
                "p (j h c) -> p j h c", j=4, h=4, c=128)
            sq, sqk, vt = tl["sq"], tl["sqk"], tl["vt"]
            st_ = {}

            def fA():
                ssA = pPJ.tile([33, TCH], F32, tag="pj", name="ssA")
                nc.tensor.matmul(ssA[:], selb_sb[:], sq[:, 0:512],
                                 start=True, stop=True)
                ssB = pPJ.tile([33, TCH], F32, tag="pj", name="ssB")
                nc.tensor.matmul(ssB[:], selb_sb[:], sq[:, 512:1024],
                                 start=True, stop=True)
                ssk = pPJ.tile([1, TCH], F32, tag="pj", name="ssk")
                nc.tensor.matmul(ssk[:], on64_sb[:], sqk[:],
                                 start=True, stop=True)
                srtA = sm.tile([33, TCH], F32, tag="u", name="srtA")
                nc.scalar.activation(srtA[:], ssA[:], AF.Sqrt,
                                     bias=biasq_sb[0:33], scale=1.0)
                srtB = sm.tile([33, TCH], F32, tag="u", name="srtB")
                nc.scalar.activation(srtB[:], ssB[:], AF.Sqrt,
                                     bias=biasq_sb[0:33], scale=1.0)
                srtk = sm.tile([1, TCH], F32, tag="u", name="srtk")
                nc.scalar.activation(srtk[:], ssk[:], AF.Sqrt,
                                     bias=biask_sb[:], scale=1.0 / 64)
                rcpA = sm.tile([33, TCH], F32, tag="rf", name="rcpA")
                nc.vector.reciprocal_approx_fast(rcpA[:], srtA[:])
                rcpB = sm.tile([33, TCH], F32, tag="rf", name="rcpB")
                nc.vector.reciprocal_approx_fast(rcpB[:], srtB[:])
                rkf = sm.tile([1, TCH], F32, tag="rf", name="rkf")
                nc.vector.reciprocal_approx_fast(rkf[:], srtk[:])
                rcbA = sm.tile([33, TCH], BF, tag="rc", name="rcbA")
                nc.scalar.copy(rcbA[:], rcpA[:])
                rcbB = sm.tile([33, TCH], BF, tag="rc", name="rcbB")
                nc.scalar.copy(rcbB[:], rcpB[:])
                rkb = sm.tile([1, TCH], BF, tag="rc", name="rkb")
                nc.scalar.copy(rkb[:], rkf[:])
                st_.update(rcbA=rcbA, rcbB=rcbB, rkb=rkb)

            def fB():
                for p in range(2):
                    rcb = st_["rcbA"] if p == 0 else st_["rcbB"]
                    bcps = pPJ.tile([128, TCH], F32, tag="pj", name="bcps")
                    nc.tensor.matmul(bcps[:], selw_sb[:], rcb[:],
                                     start=True, stop=True)
                    pb = tl["pb0"] if p == 0 else tl["pb1"]
                    pbn = wk.tile([128, TCH], BF, tag="w", name="pbn")
                    nc.vector.tensor_mul(pbn[:], pb[:], bcps[:])
                    A = wk.tile([128, TCH], BF, tag="w", name="ropeA")
                    P2 = wk.tile([128, TCH], BF, tag="w", name="ropeP2")
                    nc.vector.tensor_mul(A[:], pbn[:], cs1_sb[:, csl])
                    nc.vector.tensor_mul(P2[:], pbn[:], cs2_sb[:, csl])
                    st_[f"A{p}"] = A
                    st_[f"P{p}"] = P2
                kbc = pPJ.tile([64, TCH], F32, tag="pj", name="kbc")
                nc.tensor.matmul(kbc[:], on1x_sb[:], st_["rkb"][:],
                                 start=True, stop=True)
                kbcs = wk.tile([64, TCH], BF, tag="w", name="kbcs")
                nc.vector.tensor_copy(kbcs[:], kbc[:])
                kb = tl["kb"]
                Ak = wk.tile([64, TCH], BF, tag="w", name="ropeAk")
                Pk = wk.tile([64, TCH], BF, tag="w", name="ropePk")
                nc.vector.tensor_mul(Ak[:], kb[:], cs1_sb[0:64, csl])
                nc.vector.tensor_mul(Pk[:], kb[:], cs2_sb[0:64, csl])
                st_.update(kbcs=kbcs, Ak=Ak, Pk=Pk)

            def fC():
                for p in range(2):
                    ro = pST.tile([128, TCH], F32, tag="st", name="ro")
                    nc.tensor.matmul(ro[:], ma_sb[:], st_[f"A{p}"][:],
                                     start=True, stop=False)
                    nc.tensor.matmul(ro[:], mb_sb[:], st_[f"P{p}"][:],
                                     start=False, stop=True)
                    for i in range(2):
                        h = 2 * p + i
                        nc.vector.tensor_copy(q4v[:, :, h, :],
                                              ro[64 * i:64 * i + 64])
                rok = pYX.tile([64, TCH], F32, tag="yx", name="rok")
                nc.tensor.matmul(rok[:], ma_sb[0:64, 0:64], st_["Ak"][:],
                                 start=True, stop=False)
                nc.tensor.matmul(rok[:], mb_sb[0:64, 0:64], st_["Pk"][:],
                                 start=False, stop=True)
                nc.vector.tensor_mul(kt_sb[:, csl], rok[:], st_["kbcs"][:])

            def fD():
                vtpk = pPJ.tile([128, 256], BF, tag="pj", name="vtpk")
                for j in range(4):
                    tt = ch * 4 + j
                    vtp = vtpk[:, j * 64:(j + 1) * 64]
                    nc.tensor.transpose(vtp, vt[:, j * 128:(j + 1) * 128],
                                        id_sb[:])
                    nc.vector.scalar_tensor_tensor(
                        vn_sb[:, tt * 65: tt * 65 + 64],
                        ve_sb[:, tt * 64:(tt + 1) * 64], g_sb[:, tt:tt + 1],
                        vtp, op0=OP.mult, op1=OP.add)

            return [fA, fB, fC, fD]

        tls = []
        stages0 = None
        for ch in range(NCH):
            tls.append(chunk_mms(ch))
            if ch == 0:
                emit_gates()
                continue
            if ch == 1:
                stages0 = epilogue_stages(0, tls[0])
            stages0[ch - 1]()
        stages0[3]()

        # ============ attention + projection, lagged pipeline ============
        # block i:  PE: st(i+1) | yext(i) | proj(i-2) | bcq(i)
        #           ACT: exp(i+1)   DVE/GPS: masks(i+1), rrf/rrb(i), yt(i)
        pt_tiles = {}   # qt -> SBUF exp tile
        rr_tiles = {}   # qt -> [1,512] bf16 recip of denominators
        yx_tiles = {}   # qt -> yext psum view
        bc_tiles = {}   # qt -> bcq psum

        def emit_st(qt):
            """Scores for q-tile qt into pool tiles + exp + masks.

            Both masks run on the (otherwise idle) gpsimd queue so their
            waits on exp completion never block the DVE queue.  The
            window-edge mask consumes exp #1 so it is emitted first."""
            lo = max(0, qt - 4)
            q_ap = q4t[:, qt * 512:(qt + 1) * 512]
            pt = ptp.tile([128, 2560], BF, tag="pt", name=f"pt{qt}")
            o0 = (4 - min(qt, 4)) * 512
            n = qt + 1 - lo
            kts = list(range(lo, qt + 1))
            groups = []
            while len(kts) >= 3 or len(kts) == 2:
                groups.append(kts[:2])
                kts = kts[2:]
            if kts:
                groups.append(kts)
            for grp in groups:
                w = len(grp) * 512
                if len(grp) == 2:
                    tl_ = pST.tile([128, 2 * TCH], F32, tag="st",
                                   name=f"st{qt}_{grp[0]}")
                else:
                    tl_ = pSS.tile([128, TCH], F32, tag="ss",
                                   name=f"st{qt}_{grp[0]}")
                for j, kt in enumerate(grp):
                    nc.tensor.matmul(tl_[:, j * 512:(j + 1) * 512],
                                     kt_sb[:, kt * 128:(kt + 1) * 128], q_ap,
                                     start=True, stop=True)
                off = o0 + (grp[0] - lo) * 512
                nc.scalar.activation(pt[:, off:off + w], tl_[:, 0:w], AF.Exp)
            nc.gpsimd.tensor_mul(pt[:, 2048:2560], pt[:, 2048:2560],
                                 mskc_sb[:])
            pt_tiles[qt] = pt

        def emit_vmask(qt):
            if qt >= 4 and qt in pt_tiles:
                pt = pt_tiles[qt]
                nc.vector.tensor_mul(pt[:, 0:512], pt[:, 0:512], mskw_sb[:])

        def emit_yext(qt):
            lo = max(0, qt - 4)
            pt = pt_tiles.pop(qt)
            yext = pYX.tile([65, TCH], F32, tag="yx", name=f"yext{qt}")
            for kt in range(lo, qt + 1):
                off = (kt - qt + 4) * 512
                nc.tensor.matmul(yext[:],
                                 vn_sb[:, kt * 65: kt * 65 + 65],
                                 pt[:, off:off + 512],
                                 start=(kt == lo), stop=(kt == qt))
            yx_tiles[qt] = yext
            # reciprocal of denominators (row 64), bf16 for the PE broadcast
            dd = sm.tile([1, TCH], F32, tag="dd", name="dd")
            nc.vector.tensor_copy(dd[:], yext[64:65, :])
            rrf = sm.tile([1, TCH], F32, tag="rf", name="rrf")
            nc.vector.reciprocal_approx_fast(rrf[:], dd[:])
            rrb = sm.tile([1, TCH], BF, tag="rrb", name="rrb")
            nc.vector.tensor_copy(rrb[:], rrf[:])
            rr_tiles[qt] = rrb

        def emit_bcq(qt):
            bcq = pPJ.tile([64, TCH], F32, tag="pj", name=f"bcq{qt}")
            nc.tensor.matmul(bcq[:], on1x_sb[:], rr_tiles.pop(qt)[:],
                             start=True, stop=True)
            bc_tiles[qt] = bcq

        def emit_yt(qt):
            """Normalize + copy yext into yt layout (2 strided DVE muls)."""
            yext, bcq = yx_tiles.pop(qt), bc_tiles.pop(qt)
            yv = yext[0:64].rearrange("p (b i c) -> p b i c", b=2, i=2, c=128)
            if TWO_PSUM:
                bsrc = bcq
            else:
                bsrc = ow.tile([64, TCH], BF, tag="bca", name="bca")
                nc.vector.tensor_copy(bsrc[:], bcq[:])
            bv = bsrc[0:64].rearrange("p (b i c) -> p b i c", b=2, i=2, c=128)
            for i in range(2):
                ytv = yt_sb[64 * i: 64 * i + 64].rearrange(
                    "p (b t) -> p b t", b=2, t=T)
                nc.vector.tensor_mul(ytv[:, :, qt * 128:(qt + 1) * 128],
                                     yv[:, :, i, :], bv[:, :, i, :])

        def emit_proj(pq):
            for cc in range(2):
                ops = pPJ.tile([128, TCH], F32, tag="pj", name=f"ops{pq}_{cc}")
                for p in range(2):
                    nc.tensor.matmul(
                        ops[:],
                        yt_sb[:, p * T + pq * 128: p * T + (pq + 1) * 128],
                        wp_sb[:, p * 1024 + cc * 512: p * 1024 + cc * 512 + 512],
                        start=(p == 0), stop=(p == 1))
                o_sb = ow.tile([128, TCH], BF, tag="o", name="osb")
                if cc == 0:
                    nc.scalar.copy(o_sb[:], ops[:])
                else:
                    nc.vector.tensor_copy(o_sb[:], ops[:])
                nc.sync.dma_start(
                    out_d[pq * 128:(pq + 1) * 128, cc * 512:(cc + 1) * 512],
                    o_sb[:])

        # block i: PE order  bcq(i-1) | st(i+1) | yext(i) | proj(i-2)
        #          ACT: exp(i+1) (+osb cc0 of i-2 at tail)
        #          DVE: bca/yt(i-1), rrf/rrb(i), osb cc1 (i-2)
        #          GPS: vmask(i+1), diag mask(i+1)

        def emit_block(i):
            if i + 1 < NTT:
                emit_st(i + 1)
            if i < NTT:
                emit_yext(i)
            if i + 1 < NTT:
                emit_vmask(i + 1)
            if 0 <= i - 2 < NTT:
                emit_proj(i - 2)
            if i < NTT:
                emit_bcq(i)
                emit_yt(i)

        tls = []
        stages0 = None
        for ch in range(NCH):
            tls.append(chunk_mms(ch))
            if ch == 0:
                emit_gates()
                continue
            if ch == 1:
                stages0 = epilogue_stages(0, tls[0])
            stages0[ch - 1]()
        stages0[3]()

        # ============ attention + projection, lagged pipeline ============
        # block i:  PE: st(i+1) | yext(i) | proj(i-2) | bcq(i)
        #           ACT: exp(i+1)   DVE/GPS: masks(i+1), rrf/rrb(i), yt(i)
        pt_tiles = {}   # qt -> SBUF exp tile
        rr_tiles = {}   # qt -> [1,512] bf16 recip of denominators
        yx_tiles = {}   # qt -> yext psum view
        bc_tiles = {}   # qt -> bcq psum

        def emit_st(qt):
            """Scores for q-tile qt into pool tiles + exp + masks.

            Both masks run on the (otherwise idle) gpsimd queue so their
            waits on exp completion never block the DVE queue.  The
            window-edge mask consumes exp #1 so it is emitted first."""
            lo = max(0, qt - 4)
            q_ap = q4t[:, qt * 512:(qt + 1) * 512]
            pt = ptp.tile([128, 2560], BF, tag="pt", name=f"pt{qt}")
            o0 = (4 - min(qt, 4)) * 512
            n = qt + 1 - lo
            kts = list(range(lo, qt + 1))
            groups = []
            while len(kts) >= 3 or len(kts) == 2:
                groups.append(kts[:2])
                kts = kts[2:]
            if kts:
                groups.append(kts)
            for grp in groups:
                w = len(grp) * 512
                if len(grp) == 2:
                    tl_ = pST.tile([128, 2 * TCH], F32, tag="st",
                                   name=f"st{qt}_{grp[0]}")
                else:
                    tl_ = pSS.tile([128, TCH], F32, tag="ss",
                                   name=f"st{qt}_{grp[0]}")
                for j, kt in enumerate(grp):
                    nc.tensor.matmul(tl_[:, j * 512:(j + 1) * 512],
                                     kt_sb[:, kt * 128:(kt + 1) * 128], q_ap,
                                     start=True, stop=True)
                off = o0 + (grp[0] - lo) * 512
                nc.scalar.activation(pt[:, off:off + w], tl_[:, 0:w], AF.Exp)
            nc.gpsimd.tensor_mul(pt[:, 2048:2560], pt[:, 2048:2560],
                                 mskc_sb[:])
            pt_tiles[qt] = pt

        def emit_vmask(qt):
            if qt >= 4 and qt in pt_tiles:
                pt = pt_tiles[qt]
                nc.vector.tensor_mul(pt[:, 0:512], pt[:, 0:512], mskw_sb[:])

        def emit_yext(qt):
            lo = max(0, qt - 4)
            pt = pt_tiles.pop(qt)
            yext = pYX.tile([65, TCH], F32, tag="yx", name=f"yext{qt}")
            for kt in range(lo, qt + 1):
                off = (kt - qt + 4) * 512
                nc.tensor.matmul(yext[:],
                                 vn_sb[:, kt * 65: kt * 65 + 65],
                                 pt[:, off:off + 512],
                                 start=(kt == lo), stop=(kt == qt))
            yx_tiles[qt] = yext
            # reciprocal of denominators (row 64), bf16 for the PE broadcast
            dd = sm.tile([1, TCH], F32, tag="dd", name="dd")
            nc.vector.tensor_copy(dd[:], yext[64:65, :])
            rrf = sm.tile([1, TCH], F32, tag="rf", name="rrf")
            nc.vector.reciprocal_approx_fast(rrf[:], dd[:])
            rrb = sm.tile([1, TCH], BF, tag="rrb", name="rrb")
            nc.vector.tensor_copy(rrb[:], rrf[:])
            rr_tiles[qt] = rrb

        def emit_bcq(qt):
            bcq = pPJ.tile([64, TCH], F32, tag="pj", name=f"bcq{qt}")
            nc.tensor.matmul(bcq[:], on1x_sb[:], rr_tiles.pop(qt)[:],
                             start=True, stop=True)
            bc_tiles[qt] = bcq

        def emit_yt(qt):
            """Normalize + copy yext into yt layout (2 strided DVE muls)."""
            yext, bcq = yx_tiles.pop(qt), bc_tiles.pop(qt)
            yv = yext[0:64].rearrange("p (b i c) -> p b i c", b=2, i=2, c=128)
            if TWO_PSUM:
                bsrc = bcq
            else:
                bsrc = ow.tile([64, TCH], BF, tag="bca", name="bca")
                nc.vector.tensor_copy(bsrc[:], bcq[:])
            bv = bsrc[0:64].rearrange("p (b i c) -> p b i c", b=2, i=2, c=128)
            for i in range(2):
                ytv = yt_sb[64 * i: 64 * i + 64].rearrange(
                    "p (b t) -> p b t", b=2, t=T)
                nc.vector.tensor_mul(ytv[:, :, qt * 128:(qt + 1) * 128],
                                     yv[:, :, i, :], bv[:, :, i, :])

        def emit_proj(pq):
            for cc in range(2):
                ops = pPJ.tile([128, TCH], F32, tag="pj", name=f"ops{pq}_{cc}")
                for p in range(2):
                    nc.tensor.matmul(
                        ops[:],
                        yt_sb[:, p * T + pq * 128: p * T + (pq + 1) * 128],
                        wp_sb[:, p * 1024 + cc * 512: p * 1024 + cc * 512 + 512],
                        start=(p == 0), stop=(p == 1))
                o_sb = ow.tile([128, TCH], BF, tag="o", name="osb")
                if cc == 0:
                    nc.scalar.copy(o_sb[:], ops[:])
                else:
                    nc.vector.tensor_copy(o_sb[:], ops[:])
                nc.sync.dma_start(
                    out_d[pq * 128:(pq + 1) * 128, cc * 512:(cc + 1) * 512],
                    o_sb[:])

        # block i: PE order  bcq(i-1) | st(i+1) | yext(i) | proj(i-2)
        #          ACT: exp(i+1) (+osb cc0 of i-2 at tail)
        #          DVE: bca/yt(i-1), rrf/rrb(i), osb cc1 (i-2)
        #          GPS: vmask(i+1), diag mask(i+1)

        def emit_block(i):
            if i + 1 < NTT:
                emit_st(i + 1)
            if i < NTT:
                emit_yext(i)
            if i + 1 < NTT:
                emit_vmask(i + 1)
            if 0 <= i - 2 < NTT:
                emit_proj(i - 2)
            if i < NTT:
                emit_bcq(i)
                emit_yt(i)

        sts_by_ch = {ch: epilogue_stages(ch, tls[ch]) for ch in (1, 2, 3)}
        emit_st(0)
        for i in range(NTT + 2):
            emit_block(i)
            ch, s = i // 4 + 1, i % 4
            if ch <= 3 and i < 12:
                sts_by_ch[ch][s]()
    nc.compile()
    return nc


def _prep_inputs(x, ve, cos, sin, Wq, Wk, Wv, Wproj, Wgate):
    """Build the 8 per-core input maps (host-side sharding + layout prep)."""
    cosT = np.ascontiguousarray(cos.T).astype(np.float32)   # [32, T]
    sinT = np.ascontiguousarray(sin.T).astype(np.float32)
    cs1 = np.concatenate([cosT, sinT, cosT, sinT], 0).astype(BF16)  # [128, T]
    cs2 = np.concatenate([sinT, cosT, sinT, cosT], 0).astype(BF16)
    triu = np.triu(np.ones((128, 128), np.float32))
    tril = np.tril(np.ones((128, 128), np.float32))
    masks = np.concatenate([np.tile(triu, (1, 4)), np.tile(tril, (1, 4))],
                           1).astype(BF16)                  # [128, 1024]
    ident = np.eye(64, dtype=BF16)
    selq2 = np.zeros((128, 2), np.float32)
    selq2[0:64, 0] = 1.0
    selq2[64:128, 1] = 1.0
    selq2 = selq2.astype(BF16)
    # selw: rows {0,32} hold the 2->128 broadcast selectors
    selw = np.zeros((33, 128), np.float32)
    selw[0, 0:64] = 1.0
    selw[32, 64:128] = 1.0
    selw = selw.astype(BF16)
    onw = np.ones((65, 64), BF16)
    selb = np.zeros((128, 33), np.float32)
    selb[0:64, 0] = 1.0
    selb[64:128, 32] = 1.0
    selb = selb.astype(BF16)
    ones64 = np.ones((64, 1), BF16)
    ones1x64 = np.ones((1, 64), BF16)
    # rope combine shuffle matrices: ro = MA^T @ (x*cs1) + MB^T @ (x*cs2)
    MA = np.zeros((128, 128), np.float32)
    MB = np.zeros((128, 128), np.float32)
    for hh in (0, 64):
        for j in range(32):
            m = hh + j
            MA[m, m] = 1.0
            MA[m + 32, m] = -1.0
            m2 = hh + 32 + j
            MB[hh + j, m2] = 1.0
            MB[m2, m2] = 1.0
    mab = np.concatenate([MA, MB], 1).astype(BF16)

    xT = [np.ascontiguousarray(x[b].astype(BF16).T).reshape(8, 128, T)
          for b in range(B)]
    in_maps = []
    for c in range(NCORES):
        b, g = c // 4, c % 4
        wq_g = np.ascontiguousarray(np.transpose(
            Wq[:, g * 256:(g + 1) * 256].reshape(8, 128, 256),
            (1, 0, 2)).reshape(128, 8 * 256)).astype(BF16)
        wkv_g = np.ascontiguousarray(np.transpose(np.concatenate(
            [Wk[:, g * 64:(g + 1) * 64], Wv[:, g * 64:(g + 1) * 64]],
            1).reshape(8, 128, 128), (1, 0, 2)).reshape(128, 8 * 128)
        ).astype(BF16)
        wg_g = np.ascontiguousarray(Wgate[:, g:g + 1]).astype(BF16)
        wp_g = np.ascontiguousarray(np.transpose(
            Wproj[g * 256:(g + 1) * 256, :].reshape(2, 128, 1024),
            (1, 0, 2)).reshape(128, 2 * 1024)).astype(BF16)
        ve_g = np.ascontiguousarray(np.transpose(
            (2.0 * ve[b, :, g * 64:(g + 1) * 64]).reshape(16, 128, 64),
            (1, 0, 2)).reshape(128, 16 * 64)).astype(BF16)
        in_maps.append({
            "xt": xT[b], "wq": wq_g, "wkv": wkv_g, "wg": wg_g, "wp": wp_g,
            "cs1": cs1, "cs2": cs2, "ve2": ve_g, "masks": masks,
            "ident": ident, "selq2": selq2, "selw": selw, "onw": onw,
            "ones64": ones64, "selb": selb, "ones1x64": ones1x64, "mab": mab,
        })
    return in_maps


def _run(inputs, trace=False, tmpdir=None):
    if "nc" not in _cache:
        _cache["nc"] = _build()
    nc = _cache["nc"]
    in_maps = _prep_inputs(**inputs)
    res = run_bass_kernel_spmd(nc, in_maps, list(range(NCORES)), trace=trace,
                               tmpdir=tmpdir)
    out = np.zeros((B, T, N_EMBD), np.float32)
    for c in range(NCORES):
        out[c // 4] += np.asarray(res.results[c]["out"]).astype(np.float32)
    return out, res


def kernel(**inputs):
    out, _ = _run(inputs)
    return out


# revision 56
# speedup vs baseline: 1.0439x; 1.0439x over previous
"""Trainium2 Bass kernel for sliding-window causal self-attention (GQA + RoPE +
RMS-norm QK + value-embedding gating).

Sharding: 8 cores = 2 (batch) x 4 (KV groups).  Each core handles one batch
element and one KV head (= 4 query heads), computes a partial output through
the row-slice of Wproj for its heads; the host sums the 4 partials per batch.

v3: DMA front-loading + PE warmup (HAM un-throttle), K pre-normalized in
phase 1 so the attention exp has uniform scale and merges into 2 wide ACT
instructions per q-tile reading a single 5-bank PSUM score arena, lagged
attention pipeline (scores/exp one q-tile ahead of PV; proj two behind)
keeping the PE stream dense, mask/cast work spread to the GpSimd engine.
"""

import sys
import os

for _p in ("/root/.axon_site", "/root/.axon_site/_ro/trn_rl_repo",
           "/root/.axon_site/_ro/pypackages", "/opt/trn_rl_repo"):
    if os.path.isdir(_p) and _p not in sys.path:
        sys.path.append(_p)

import numpy as np
import ml_dtypes
from contextlib import ExitStack

import concourse.bass as bass
import concourse.tile as tile
from concourse import bacc, mybir
from concourse.bass_utils import run_bass_kernel_spmd

BF16 = ml_dtypes.bfloat16
N_HEAD, N_KV, HEAD_DIM, WINDOW, N_EMBD = 16, 4, 64, 512, 1024
B, T = 2, 2048
NCORES = 8
TCH = 512               # token chunk for the projection phase
NCH = T // TCH          # 4
NTT = T // 128          # 16 t-tiles

F32 = mybir.dt.float32
BF = mybir.dt.bfloat16
AF = mybir.ActivationFunctionType
OP = mybir.AluOpType

TWO_PSUM = False        # DVE tensor ops reading two PSUM operands
WIDE_EXP = False        # ACT exp reads spanning multiple PSUM banks

_cache = {}


def _build():
    nc = bacc.Bacc("TRN2", target_bir_lowering=False, debug=False,
                   num_devices=NCORES)

    xt_d = nc.dram_tensor("xt", [8, 128, T], BF, kind="ExternalInput")
    wq_d = nc.dram_tensor("wq", [128, 8 * 256], BF, kind="ExternalInput")
    wkv_d = nc.dram_tensor("wkv", [128, 8 * 128], BF, kind="ExternalInput")
    wg_d = nc.dram_tensor("wg", [32, 1], BF, kind="ExternalInput")
    wp_d = nc.dram_tensor("wp", [128, 2 * 1024], BF, kind="ExternalInput")
    cs1_d = nc.dram_tensor("cs1", [128, T], BF, kind="ExternalInput")
    cs2_d = nc.dram_tensor("cs2", [128, T], BF, kind="ExternalInput")
    ve_d = nc.dram_tensor("ve2", [128, 16 * 64], BF, kind="ExternalInput")
    msk_d = nc.dram_tensor("masks", [128, 1024], BF, kind="ExternalInput")
    id_d = nc.dram_tensor("ident", [64, 64], BF, kind="ExternalInput")
    selq_d = nc.dram_tensor("selq2", [128, 2], BF, kind="ExternalInput")
    selw_d = nc.dram_tensor("selw", [33, 128], BF, kind="ExternalInput")
    on64_d = nc.dram_tensor("ones64", [64, 1], BF, kind="ExternalInput")
    selb_d = nc.dram_tensor("selb", [128, 33], BF, kind="ExternalInput")
    onw_d = nc.dram_tensor("onw", [65, 64], BF, kind="ExternalInput")
    on1x_d = nc.dram_tensor("ones1x64", [1, 64], BF, kind="ExternalInput")
    mab_d = nc.dram_tensor("mab", [128, 256], BF, kind="ExternalInput")
    out_d = nc.dram_tensor("out", [T, N_EMBD], BF, kind="ExternalOutput")

    with tile.TileContext(nc) as tc, ExitStack() as ctx:
        pers = ctx.enter_context(tc.tile_pool(name="pers", bufs=1))
        wk = ctx.enter_context(tc.tile_pool(name="wk", bufs=8))
        wfp = ctx.enter_context(tc.tile_pool(name="wfp", bufs=4))
        sm = ctx.enter_context(tc.tile_pool(name="sm", bufs=4))
        ow = ctx.enter_context(tc.tile_pool(name="ow", bufs=4))
        evp = ctx.enter_context(tc.tile_pool(name="evp", bufs=20))
        sqp = ctx.enter_context(tc.tile_pool(name="sqp", bufs=4))
        ptp = ctx.enter_context(tc.tile_pool(name="ptp", bufs=3))
        # PSUM: score tiles (5 banks) + yx (1 bank) + pj pool (2 banks)
        pST = ctx.enter_context(tc.tile_pool(name="pST", bufs=5, space="PSUM"))
        pYX = ctx.enter_context(tc.tile_pool(name="pYX", bufs=1, space="PSUM"))
        pPJ = ctx.enter_context(tc.tile_pool(name="pPJ", bufs=2, space="PSUM"))

        # ---- DMA front-loading: ident first (PE warmup dep), then xt/w
        # in first-use order; small aux tensors go on the vector queue ----
        id_sb = pers.tile([64, 64], BF, tag="ident")
        nc.sync.dma_start(id_sb[:], id_d[:])
        xt_sb = [None] * 8
        for k, eng in ((0, nc.sync), (1, nc.gpsimd), (2, nc.scalar)):
            xt_sb[k] = pers.tile([128, T], BF, tag=f"xt{k}", name=f"xt{k}")
        wq_sb = pers.tile([128, 8 * 256], BF, tag="wq")
        nc.scalar.dma_start(wq_sb[:], wq_d[:])
        cs1_sb = pers.tile([128, T], BF, tag="cs1")
        nc.gpsimd.dma_start(cs1_sb[:], cs1_d[:])
        nc.sync.dma_start(xt_sb[0][:], xt_d[0])
        nc.gpsimd.dma_start(xt_sb[1][:], xt_d[1])
        nc.scalar.dma_start(xt_sb[2][:], xt_d[2])
        for k, eng in ((3, nc.sync), (4, nc.gpsimd), (5, nc.scalar),
                       (6, nc.sync), (7, nc.gpsimd)):
            t_ = pers.tile([128, T], BF, tag=f"xt{k}", name=f"xt{k}")
            eng.dma_start(t_[:], xt_d[k])
            xt_sb[k] = t_
        wkv_sb = pers.tile([128, 8 * 128], BF, tag="wkv")
        nc.scalar.dma_start(wkv_sb[:], wkv_d[:])
        selq_sb = pers.tile([128, 2], BF, tag="selq2")
        nc.sync.dma_start(selq_sb[:], selq_d[:])
        selw_sb = pers.tile([33, 128], BF, tag="selw")
        nc.sync.dma_start(selw_sb[:], selw_d[:])
        on64_sb = pers.tile([64, 1], BF, tag="on64")
        nc.sync.dma_start(on64_sb[:], on64_d[:])
        selb_sb = pers.tile([128, 33], BF, tag="selb")
        nc.sync.dma_start(selb_sb[:], selb_d[:])
        onw_sb = pers.tile([65, 64], BF, tag="onw")
        nc.sync.dma_start(onw_sb[:], onw_d[:])
        on1x_sb = pers.tile([1, 64], BF, tag="on1x")
        nc.sync.dma_start(on1x_sb[:], on1x_d[:])
        wg_sb = pers.tile([32, 1], BF, tag="wg")
        nc.sync.dma_start(wg_sb[:], wg_d[:])
        mab_sb_ = pers.tile([128, 256], BF, tag="mab")
        nc.sync.dma_start(mab_sb_[:], mab_d[:])
        ma_sb = mab_sb_[:, 0:128]
        mb_sb = mab_sb_[:, 128:256]
        cs2_sb = pers.tile([128, T], BF, tag="cs2")
        nc.gpsimd.dma_start(cs2_sb[:], cs2_d[:])
        ve_sb = pers.tile([128, 16 * 64], BF, tag="ve")
        nc.sync.dma_start(ve_sb[:], ve_d[:])
        mskc_sb = pers.tile([128, 512], BF, tag="mskc")
        nc.sync.dma_start(mskc_sb[:], msk_d[:, 0:512])
        mskw_sb = pers.tile([128, 512], BF, tag="mskw")
        nc.sync.dma_start(mskw_sb[:], msk_d[:, 512:1024])
        wp_sb = pers.tile([128, 2 * 1024], BF, tag="wp")
        nc.scalar.dma_start(wp_sb[:], wp_d[:])

        # ---- persistent intermediates ----
        # Q^T, 4 heads side-by-side per q-tile: [64, qt(16) x h(4) x 128]
        q4t = pers.tile([64, NTT * 512], BF, tag="q4t")
        kt_sb = pers.tile([64, T], BF, tag="kt")      # K^T (pre-normalized)
        vn_sb = pers.tile([128, NTT * 65], BF, tag="vn")  # V natural + ones col
        # y^T: [128 (2 heads stacked), p(2) x T]
        yt_sb = pers.tile([128, 2 * T], BF, tag="yt")
        g_sb = pers.tile([128, NTT], F32, tag="g")    # sigmoid gates, natural

        nc.vector.memset(vn_sb[:], 1.0)      # ones columns (col 64 of each group)
        biasq_sb = pers.tile([33, 1], F32, tag="biasq")
        nc.vector.memset(biasq_sb[:], 64e-6)
        biask_sb = pers.tile([1, 1], F32, tag="biask")
        nc.vector.memset(biask_sb[:], 1e-6)

        # ---- PE warmup during initial DMA (HAM un-throttle); depends only
        # on the tiny ident tensor which is the first DMA issued ----
        warm = pPJ.tile([128, TCH], F32, tag="pj", name="warm")
        for _ in range(40):
            nc.tensor.matmul(warm[0:64, 0:64], id_sb[:], id_sb[:],
                             start=True, stop=True)

        def emit_gates():
            # gates for all t-tiles (sigmoid via exp + reciprocal)
            gps = pPJ.tile([128, NTT], F32, tag="pj", name="gps")
            for tt in range(NTT):
                nc.tensor.matmul(gps[:, tt:tt + 1],
                                 xt_sb[0][0:32, tt * 128:(tt + 1) * 128],
                                 wg_sb[:], start=True, stop=True)
            eg = sm.tile([128, NTT], F32, tag="u", name="eg")
            nc.scalar.activation(eg[:], gps[:], AF.Exp, scale=-1.0)
            eg1 = sm.tile([128, NTT], F32, tag="u", name="eg1")
            nc.vector.tensor_scalar_add(eg1[:], eg[:], 1.0)
            nc.vector.reciprocal(g_sb[:], eg1[:])

        def qkv_matmul(psum, w_sb, col0, ncol, c0):
            for k in range(8):
                nc.tensor.matmul(
                    psum, w_sb[:, k * ncol + col0: k * ncol + col0 + 128],
                    xt_sb[k][:, c0:c0 + TCH],
                    start=(k == 0), stop=(k == 7))

        def chunk_mms(ch):
            """QKV matmuls (dense PE block) + immediate PSUM evacuation."""
            c0 = ch * TCH
            psq0 = pST.tile([128, TCH], F32, tag="st", name="psq0")
            qkv_matmul(psq0[:], wq_sb, 0, 256, c0)
            psq1 = pST.tile([128, TCH], F32, tag="st", name="psq1")
            qkv_matmul(psq1[:], wq_sb, 128, 256, c0)
            pskv = pST.tile([128, TCH], F32, tag="st", name="pskv")
            qkv_matmul(pskv[:], wkv_sb, 0, 128, c0)
            tl = {}
            sq = sqp.tile([128, 1024], BF, tag="sq", name="sq")
            nc.scalar.square(sq[:, 0:512], psq0[:])
            nc.scalar.square(sq[:, 512:1024], psq1[:])
            pb0 = evp.tile([128, TCH], BF, tag="e", name="pb0")
            nc.scalar.copy(pb0[:], psq0[:])
            pb1 = evp.tile([128, TCH], BF, tag="e", name="pb1")
            nc.vector.tensor_copy(pb1[:], psq1[:])
            sqk = evp.tile([64, TCH], BF, tag="e", name="sqk")
            nc.scalar.square(sqk[:], pskv[0:64])
            kb = evp.tile([64, TCH], BF, tag="e", name="kb")
            nc.vector.tensor_copy(kb[:], pskv[0:64])
            vt = evp.tile([64, TCH], BF, tag="e", name="vt")
            nc.vector.tensor_copy(vt[:], pskv[64:128])
            tl.update(sq=sq, pb0=pb0, pb1=pb1, sqk=sqk, kb=kb, vt=vt)
            return tl

        def epilogue_stages(ch, tl):
            """Four emit-stages of the per-chunk rms/rope/V epilogue; the
            cross-engine chain between consecutive stages resolves while a
            full attention block sits between them in the engine queues."""
            c0 = ch * TCH
            csl = slice(c0, c0 + TCH)
            q4v = q4t[:, ch * 2048:(ch + 1) * 2048].rearrange(
                "p (j h c) -> p j h c", j=4, h=4, c=128)
            sq, sqk, vt = tl["sq"], tl["sqk"], tl["vt"]
            st_ = {}

            def fA():
                ssA = pPJ.tile([33, TCH], F32, tag="pj", name="ssA")
                nc.tensor.matmul(ssA[:], selb_sb[:], sq[:, 0:512],
                                 start=True, stop=True)
                ssB = pPJ.tile([33, TCH], F32, tag="pj", name="ssB")
                nc.tensor.matmul(ssB[:], selb_sb[:], sq[:, 512:1024],
                                 start=True, stop=True)
                ssk = pPJ.tile([1, TCH], F32, tag="pj", name="ssk")
                nc.tensor.matmul(ssk[:], on64_sb[:], sqk[:],
                                 start=True, stop=True)
                srtA = sm.tile([33, TCH], F32, tag="u", name="srtA")
                nc.scalar.activation(srtA[:], ssA[:], AF.Sqrt,
                                     bias=biasq_sb[0:33], scale=1.0)
                srtB = sm.tile([33, TCH], F32, tag="u", name="srtB")
                nc.scalar.activation(srtB[:], ssB[:], AF.Sqrt,
                                     bias=biasq_sb[0:33], scale=1.0)
                srtk = sm.tile([1, TCH], F32, tag="u", name="srtk")
                nc.scalar.activation(srtk[:], ssk[:], AF.Sqrt,
                                     bias=biask_sb[:], scale=1.0 / 64)
                rcpA = sm.tile([33, TCH], F32, tag="rf", name="rcpA")
                nc.vector.reciprocal_approx_fast(rcpA[:], srtA[:])
                rcpB = sm.tile([33, TCH], F32, tag="rf", name="rcpB")
                nc.vector.reciprocal_approx_fast(rcpB[:], srtB[:])
                rkf = sm.tile([1, TCH], F32, tag="rf", name="rkf")
                nc.vector.reciprocal_approx_fast(rkf[:], srtk[:])
                rcbA = sm.tile([33, TCH], BF, tag="rc", name="rcbA")
                nc.scalar.copy(rcbA[:], rcpA[:])
                rcbB = sm.tile([33, TCH], BF, tag="rc", name="rcbB")
                nc.scalar.copy(rcbB[:], rcpB[:])
                rkb = sm.tile([1, TCH], BF, tag="rc", name="rkb")
                nc.scalar.copy(rkb[:], rkf[:])
                st_.update(rcbA=rcbA, rcbB=rcbB, rkb=rkb)

            def fB():
                for p in range(2):
                    rcb = st_["rcbA"] if p == 0 else st_["rcbB"]
                    bcps = pPJ.tile([128, TCH], F32, tag="pj", name="bcps")
                    nc.tensor.matmul(bcps[:], selw_sb[:], rcb[:],
                                     start=True, stop=True)
                    pb = tl["pb0"] if p == 0 else tl["pb1"]
                    pbn = wk.tile([128, TCH], BF, tag="w", name="pbn")
                    nc.vector.tensor_mul(pbn[:], pb[:], bcps[:])
                    A = wk.tile([128, TCH], BF, tag="w", name="ropeA")
                    P2 = wk.tile([128, TCH], BF, tag="w", name="ropeP2")
                    nc.vector.tensor_mul(A[:], pbn[:], cs1_sb[:, csl])
                    nc.vector.tensor_mul(P2[:], pbn[:], cs2_sb[:, csl])
                    st_[f"A{p}"] = A
                    st_[f"P{p}"] = P2
                kbc = pPJ.tile([64, TCH], F32, tag="pj", name="kbc")
                nc.tensor.matmul(kbc[:], on1x_sb[:], st_["rkb"][:],
                                 start=True, stop=True)
                kbcs = wk.tile([64, TCH], BF, tag="w", name="kbcs")
                nc.vector.tensor_copy(kbcs[:], kbc[:])
                kb = tl["kb"]
                Ak = wk.tile([64, TCH], BF, tag="w", name="ropeAk")
                Pk = wk.tile([64, TCH], BF, tag="w", name="ropePk")
                nc.vector.tensor_mul(Ak[:], kb[:], cs1_sb[0:64, csl])
                nc.vector.tensor_mul(Pk[:], kb[:], cs2_sb[0:64, csl])
                st_.update(kbcs=kbcs, Ak=Ak, Pk=Pk)

            def fC():
                for p in range(2):
                    ro = pST.tile([128, TCH], F32, tag="st", name="ro")
                    nc.tensor.matmul(ro[:], ma_sb[:], st_[f"A{p}"][:],
                                     start=True, stop=False)
                    nc.tensor.matmul(ro[:], mb_sb[:], st_[f"P{p}"][:],
                                     start=False, stop=True)
                    for i in range(2):
                        h = 2 * p + i
                        nc.vector.tensor_copy(q4v[:, :, h, :],
                                              ro[64 * i:64 * i + 64])
                rok = pYX.tile([64, TCH], F32, tag="yx", name="rok")
                nc.tensor.matmul(rok[:], ma_sb[0:64, 0:64], st_["Ak"][:],
                                 start=True, stop=False)
                nc.tensor.matmul(rok[:], mb_sb[0:64, 0:64], st_["Pk"][:],
                                 start=False, stop=True)
                nc.vector.tensor_mul(kt_sb[:, csl], rok[:], st_["kbcs"][:])

            def fD():
                vtpk = pPJ.tile([128, 256], BF, tag="pj", name="vtpk")
                for j in range(4):
                    tt = ch * 4 + j
                    vtp = vtpk[:, j * 64:(j + 1) * 64]
                    nc.tensor.transpose(vtp, vt[:, j * 128:(j + 1) * 128],
                                        id_sb[:])
                    nc.vector.scalar_tensor_tensor(
                        vn_sb[:, tt * 65: tt * 65 + 64],
                        ve_sb[:, tt * 64:(tt + 1) * 64], g_sb[:, tt:tt + 1],
                        vtp, op0=OP.mult, op1=OP.add)

            return [fA, fB, fC, fD]

        tls = []
        stages0 = None
        for ch in range(NCH):
            tls.append(chunk_mms(ch))
            if ch == 0:
                emit_gates()
                continue
            if ch == 1:
                stages0 = epilogue_stages(0, tls[0])
            stages0[ch - 1]()
        stages0[3]()

        # ============ attention + projection, lagged pipeline ============
        # block i:  PE: st(i+1) | yext(i) | proj(i-2) | bcq(i)
        #           ACT: exp(i+1)   DVE/GPS: masks(i+1), rrf/rrb(i), yt(i)
        pt_tiles = {}   # qt -> SBUF exp tile
        rr_tiles = {}   # qt -> [1,512] bf16 recip of denominators
        yx_tiles = {}   # qt -> yext psum view
        bc_tiles = {}   # qt -> bcq psum

        def emit_st(qt):
            """Scores for q-tile qt into pool tiles + exp + masks.

            Both masks run on the (otherwise idle) gpsimd queue so their
            waits on exp completion never block the DVE queue.  The
            window-edge mask consumes exp #1 so it is emitted first."""
            lo = max(0, qt - 4)
            q_ap = q4t[:, qt * 512:(qt + 1) * 512]
            sts = []
            for kt in range(lo, qt + 1):
                st = pST.tile([128, TCH], F32, tag="st", name=f"st{qt}_{kt}")
                nc.tensor.matmul(st[:],
                                 kt_sb[:, kt * 128:(kt + 1) * 128], q_ap,
                                 start=True, stop=True)
                sts.append(st)
            # exp into SBUF bf16 (uniform scale: K is pre-normalized)
            pt = ptp.tile([128, 2560], BF, tag="pt", name=f"pt{qt}")
            o0 = (4 - min(qt, 4)) * 512
            for j, st in enumerate(sts):
                off = o0 + j * 512
                nc.scalar.activation(pt[:, off:off + 512], st[:], AF.Exp)
            nc.gpsimd.tensor_mul(pt[:, 2048:2560], pt[:, 2048:2560],
                                 mskc_sb[:])
            pt_tiles[qt] = pt

        def emit_vmask(qt):
            if qt >= 4 and qt in pt_tiles:
                pt = pt_tiles[qt]
                nc.vector.tensor_mul(pt[:, 0:512], pt[:, 0:512], mskw_sb[:])

        def emit_yext(qt):
            lo = max(0, qt - 4)
            pt = pt_tiles.pop(qt)
            yext = pYX.tile([65, TCH], F32, tag="yx", name=f"yext{qt}")
            for kt in range(lo, qt + 1):
                off = (kt - qt + 4) * 512
                nc.tensor.matmul(yext[:],
                                 vn_sb[:, kt * 65: kt * 65 + 65],
                                 pt[:, off:off + 512],
                                 start=(kt == lo), stop=(kt == qt))
            yx_tiles[qt] = yext
            # reciprocal of denominators (row 64), bf16 for the PE broadcast
            dd = sm.tile([1, TCH], F32, tag="dd", name="dd")
            nc.vector.tensor_copy(dd[:], yext[64:65, :])
            rrf = sm.tile([1, TCH], F32, tag="rf", name="rrf")
            nc.vector.reciprocal_approx_fast(rrf[:], dd[:])
            rrb = sm.tile([1, TCH], BF, tag="rrb", name="rrb")
            nc.vector.tensor_copy(rrb[:], rrf[:])
            rr_tiles[qt] = rrb

        def emit_bcq(qt):
            bcq = pPJ.tile([64, TCH], F32, tag="pj", name=f"bcq{qt}")
            nc.tensor.matmul(bcq[:], on1x_sb[:], rr_tiles.pop(qt)[:],
                             start=True, stop=True)
            bc_tiles[qt] = bcq

        def emit_yt(qt):
            """Normalize + copy yext into yt layout (2 strided DVE muls)."""
            yext, bcq = yx_tiles.pop(qt), bc_tiles.pop(qt)
            yv = yext[0:64].rearrange("p (b i c) -> p b i c", b=2, i=2, c=128)
            if TWO_PSUM:
                bsrc = bcq
            else:
                bsrc = ow.tile([64, TCH], BF, tag="bca", name="bca")
                nc.vector.tensor_copy(bsrc[:], bcq[:])
            bv = bsrc[0:64].rearrange("p (b i c) -> p b i c", b=2, i=2, c=128)
            for i in range(2):
                ytv = yt_sb[64 * i: 64 * i + 64].rearrange(
                    "p (b t) -> p b t", b=2, t=T)
                nc.vector.tensor_mul(ytv[:, :, qt * 128:(qt + 1) * 128],
                                     yv[:, :, i, :], bv[:, :, i, :])

        def emit_proj(pq):
            for cc in range(2):
                ops = pPJ.tile([128, TCH], F32, tag="pj", name=f"ops{pq}_{cc}")
                for p in range(2):
                    nc.tensor.matmul(
                        ops[:],
                        yt_sb[:, p * T + pq * 128: p * T + (pq + 1) * 128],
                        wp_sb[:, p * 1024 + cc * 512: p * 1024 + cc * 512 + 512],
                        start=(p == 0), stop=(p == 1))
                o_sb = ow.tile([128, TCH], BF, tag="o", name="osb")
                if cc == 0:
                    nc.scalar.copy(o_sb[:], ops[:])
                else:
                    nc.vector.tensor_copy(o_sb[:], ops[:])
                nc.sync.dma_start(
                    out_d[pq * 128:(pq + 1) * 128, cc * 512:(cc + 1) * 512],
                    o_sb[:])

        # block i: PE order  bcq(i-1) | st(i+1) | yext(i) | proj(i-2)
        #          ACT: exp(i+1) (+osb cc0 of i-2 at tail)
        #          DVE: bca/yt(i-1), rrf/rrb(i), osb cc1 (i-2)
        #          GPS: vmask(i+1), diag mask(i+1)

        def emit_block(i):
            if i + 1 < NTT:
                emit_st(i + 1)
            if i < NTT:
                emit_yext(i)
            if i + 1 < NTT:
                emit_vmask(i + 1)
            if 0 <= i - 2 < NTT:
                emit_proj(i - 2)
            if i < NTT:
                emit_bcq(i)
                emit_yt(i)

        tls = []
        stages0 = None
        for ch in range(NCH):
            tls.append(chunk_mms(ch))
            if ch == 0:
                emit_gates()
                continue
            if ch == 1:
                stages0 = epilogue_stages(0, tls[0])
            stages0[ch - 1]()
        stages0[3]()

        # ============ attention + projection, lagged pipeline ============
        # block i:  PE: st(i+1) | yext(i) | proj(i-2) | bcq(i)
        #           ACT: exp(i+1)   DVE/GPS: masks(i+1), rrf/rrb(i), yt(i)
        pt_tiles = {}   # qt -> SBUF exp tile
        rr_tiles = {}   # qt -> [1,512] bf16 recip of denominators
        yx_tiles = {}   # qt -> yext psum view
        bc_tiles = {}   # qt -> bcq psum

        def emit_st(qt):
            """Scores for q-tile qt into pool tiles + exp + masks.

            Both masks run on the (otherwise idle) gpsimd queue so their
            waits on exp completion never block the DVE queue.  The
            window-edge mask consumes exp #1 so it is emitted first."""
            lo = max(0, qt - 4)
            q_ap = q4t[:, qt * 512:(qt + 1) * 512]
            sts = []
            for kt in range(lo, qt + 1):
                st = pST.tile([128, TCH], F32, tag="st", name=f"st{qt}_{kt}")
                nc.tensor.matmul(st[:],
                                 kt_sb[:, kt * 128:(kt + 1) * 128], q_ap,
                                 start=True, stop=True)
                sts.append(st)
            # exp into SBUF bf16 (uniform scale: K is pre-normalized)
            pt = ptp.tile([128, 2560], BF, tag="pt", name=f"pt{qt}")
            o0 = (4 - min(qt, 4)) * 512
            for j, st in enumerate(sts):
                off = o0 + j * 512
                nc.scalar.activation(pt[:, off:off + 512], st[:], AF.Exp)
            nc.gpsimd.tensor_mul(pt[:, 2048:2560], pt[:, 2048:2560],
                                 mskc_sb[:])
            pt_tiles[qt] = pt

        def emit_vmask(qt):
            if qt >= 4 and qt in pt_tiles:
                pt = pt_tiles[qt]
                nc.vector.tensor_mul(pt[:, 0:512], pt[:, 0:512], mskw_sb[:])

        def emit_yext(qt):
            lo = max(0, qt - 4)
            pt = pt_tiles.pop(qt)
            yext = pYX.tile([65, TCH], F32, tag="yx", name=f"yext{qt}")
            for kt in range(lo, qt + 1):
                off = (kt - qt + 4) * 512
                nc.tensor.matmul(yext[:],
                                 vn_sb[:, kt * 65: kt * 65 + 65],
                                 pt[:, off:off + 512],
                                 start=(kt == lo), stop=(kt == qt))
            yx_tiles[qt] = yext
            # reciprocal of denominators (row 64), bf16 for the PE broadcast
            dd = sm.tile([1, TCH], F32, tag="dd", name="dd")
            nc.vector.tensor_copy(dd[:], yext[64:65, :])
            rrf = sm.tile([1, TCH], F32, tag="rf", name="rrf")
            nc.vector.reciprocal_approx_fast(rrf[:], dd[:])
            rrb = sm.tile([1, TCH], BF, tag="rrb", name="rrb")
            nc.vector.tensor_copy(rrb[:], rrf[:])
            rr_tiles[qt] = rrb

        def emit_bcq(qt):
            bcq = pPJ.tile([64, TCH], F32, tag="pj", name=f"bcq{qt}")
            nc.tensor.matmul(bcq[:], on1x_sb[:], rr_tiles.pop(qt)[:],
                             start=True, stop=True)
            bc_tiles[qt] = bcq

        def emit_yt(qt):
            """Normalize + copy yext into yt layout (2 strided DVE muls)."""
            yext, bcq = yx_tiles.pop(qt), bc_tiles.pop(qt)
            yv = yext[0:64].rearrange("p (b i c) -> p b i c", b=2, i=2, c=128)
            if TWO_PSUM:
                bsrc = bcq
            else:
                bsrc = ow.tile([64, TCH], BF, tag="bca", name="bca")
                nc.vector.tensor_copy(bsrc[:], bcq[:])
            bv = bsrc[0:64].rearrange("p (b i c) -> p b i c", b=2, i=2, c=128)
            for i in range(2):
                ytv = yt_sb[64 * i: 64 * i + 64].rearrange(
                    "p (b t) -> p b t", b=2, t=T)
                nc.vector.tensor_mul(ytv[:, :, qt * 128:(qt + 1) * 128],
                                     yv[:, :, i, :], bv[:, :, i, :])

        def emit_proj(pq):
            for cc in range(2):
                ops = pPJ.tile([128, TCH], F32, tag="pj", name=f"ops{pq}_{cc}")
                for p in range(2):
                    nc.tensor.matmul(
                        ops[:],
                        yt_sb[:, p * T + pq * 128: p * T + (pq + 1) * 128],
                        wp_sb[:, p * 1024 + cc * 512: p * 1024 + cc * 512 + 512],
                        start=(p == 0), stop=(p == 1))
                o_sb = ow.tile([128, TCH], BF, tag="o", name="osb")
                if cc == 0:
                    nc.scalar.copy(o_sb[:], ops[:])
                else:
                    nc.vector.tensor_copy(o_sb[:], ops[:])
                nc.sync.dma_start(
                    out_d[pq * 128:(pq + 1) * 128, cc * 512:(cc + 1) * 512],
                    o_sb[:])

        # block i: PE order  bcq(i-1) | st(i+1) | yext(i) | proj(i-2)
        #          ACT: exp(i+1) (+osb cc0 of i-2 at tail)
        #          DVE: bca/yt(i-1), rrf/rrb(i), osb cc1 (i-2)
        #          GPS: vmask(i+1), diag mask(i+1)

        def emit_block(i):
            if i + 1 < NTT:
                emit_st(i + 1)
            if i < NTT:
                emit_yext(i)
            if i + 1 < NTT:
                emit_vmask(i + 1)
            if 0 <= i - 2 < NTT:
                emit_proj(i - 2)
            if i < NTT:
                emit_bcq(i)
                emit_yt(i)

        sts_by_ch = {ch: epilogue_stages(ch, tls[ch]) for ch in (1, 2, 3)}
        emit_st(0)
        for i in range(NTT + 2):
            emit_block(i)
            ch, s = i // 4 + 1, i % 4
            if ch <= 3 and i < 12:
                sts_by_ch[ch][s]()
    nc.compile()
    return nc


def _prep_inputs(x, ve, cos, sin, Wq, Wk, Wv, Wproj, Wgate):
    """Build the 8 per-core input maps (host-side sharding + layout prep)."""
    cosT = np.ascontiguousarray(cos.T).astype(np.float32)   # [32, T]
    sinT = np.ascontiguousarray(sin.T).astype(np.float32)
    cs1 = np.concatenate([cosT, sinT, cosT, sinT], 0).astype(BF16)  # [128, T]
    cs2 = np.concatenate([sinT, cosT, sinT, cosT], 0).astype(BF16)
    triu = np.triu(np.ones((128, 128), np.float32))
    tril = np.tril(np.ones((128, 128), np.float32))
    masks = np.concatenate([np.tile(triu, (1, 4)), np.tile(tril, (1, 4))],
                           1).astype(BF16)                  # [128, 1024]
    ident = np.eye(64, dtype=BF16)
    selq2 = np.zeros((128, 2), np.float32)
    selq2[0:64, 0] = 1.0
    selq2[64:128, 1] = 1.0
    selq2 = selq2.astype(BF16)
    # selw: rows {0,32} hold the 2->128 broadcast selectors
    selw = np.zeros((33, 128), np.float32)
    selw[0, 0:64] = 1.0
    selw[32, 64:128] = 1.0
    selw = selw.astype(BF16)
    onw = np.ones((65, 64), BF16)
    selb = np.zeros((128, 33), np.float32)
    selb[0:64, 0] = 1.0
    selb[64:128, 32] = 1.0
    selb = selb.astype(BF16)
    ones64 = np.ones((64, 1), BF16)
    ones1x64 = np.ones((1, 64), BF16)
    # rope combine shuffle matrices: ro = MA^T @ (x*cs1) + MB^T @ (x*cs2)
    MA = np.zeros((128, 128), np.float32)
    MB = np.zeros((128, 128), np.float32)
    for hh in (0, 64):
        for j in range(32):
            m = hh + j
            MA[m, m] = 1.0
            MA[m + 32, m] = -1.0
            m2 = hh + 32 + j
            MB[hh + j, m2] = 1.0
            MB[m2, m2] = 1.0
    mab = np.concatenate([MA, MB], 1).astype(BF16)

    xT = [np.ascontiguousarray(x[b].astype(BF16).T).reshape(8, 128, T)
          for b in range(B)]
    in_maps = []
    for c in range(NCORES):
        b, g = c // 4, c % 4
        wq_g = np.ascontiguousarray(np.transpose(
            Wq[:, g * 256:(g + 1) * 256].reshape(8, 128, 256),
            (1, 0, 2)).reshape(128, 8 * 256)).astype(BF16)
        wkv_g = np.ascontiguousarray(np.transpose(np.concatenate(
            [Wk[:, g * 64:(g + 1) * 64], Wv[:, g * 64:(g + 1) * 64]],
            1).reshape(8, 128, 128), (1, 0, 2)).reshape(128, 8 * 128)
        ).astype(BF16)
        wg_g = np.ascontiguousarray(Wgate[:, g:g + 1]).astype(BF16)
        wp_g = np.ascontiguousarray(np.transpose(
            Wproj[g * 256:(g + 1) * 256, :].reshape(2, 128, 1024),
            (1, 0, 2)).reshape(128, 2 * 1024)).astype(BF16)
        ve_g = np.ascontiguousarray(np.transpose(
            (2.0 * ve[b, :, g * 64:(g + 1) * 64]).reshape(16, 128, 64),
            (1, 0, 2)).reshape(128, 16 * 64)).astype(BF16)
        in_maps.append({
            "xt": xT[b], "wq": wq_g, "wkv": wkv_g, "wg": wg_g, "wp": wp_g,
            "cs1": cs1, "cs2": cs2, "ve2": ve_g, "masks": masks,
            "ident": ident, "selq2": selq2, "selw": selw, "onw": onw,
            "ones64": ones64, "selb": selb, "ones1x64": ones1x64, "mab": mab,
        })
    return in_maps


def _run(inputs, trace=False, tmpdir=None):
    if "nc" not in _cache:
        _cache["nc"] = _build()
    nc = _cache["nc"]
    in_maps = _prep_inputs(**inputs)
    res = run_bass_kernel_spmd(nc, in_maps, list(range(NCORES)), trace=trace,
                               tmpdir=tmpdir)
    out = np.zeros((B, T, N_EMBD), np.float32)
    for c in range(NCORES):
        out[c // 4] += np.asarray(res.results[c]["out"]).astype(np.float32)
    return out, res


def kernel(**inputs):
    out, _ = _run(inputs)
    return out
